# revision 19
# baseline (speedup 1.0000x reference)
"""Trainium2 Bass kernel for nn_DPFABase (DPFA knowledge-tracing attention).

Full-input contract: kernel(**inputs) takes the unsharded inputs and returns
the full [B, S] float32 output. Internally: data-parallel over batch across
8 NeuronCores (16 examples per core). Host marshaling (same class as the
beta/response-table prep) pre-normalizes the embedding table, gathers the
per-token rows, and lays them out transposed ([H, token], fp8 e4m3) so the
device kernel spends its time on the actual FLOPs: QK matmuls, softmax,
weighted sums, sigmoid.

Key structure, per example e (16 per core):
  1. One dma_start pulls TT [128(H), 1024] fp8 (cols 0..511 hist_T,
     512..1023 next_T; rows unit-norm).
  2. 4 causal-blocked QK matmuls (fp8) write ONE PSUM tile [128, 1280]
     f32, column-packed [j0:512 | j1:384 | j3:128 | j2:256] so each
     matmul stays inside a 2KB PSUM bank.
  3. ONE ACT Exp over all 1280 cols. The time-decay bias is reduced to a
     single per-partition vector -k*p + 63.5k (common to all blocks) by
     folding each block's constant decay offset exp(k*(192-128j)) into
     the host-marshaled taux columns (exact rescaling; num/den ratio is
     unchanged). Per-q decay parts cancel in softmax.
  4. Two batched causal-mask multiplies on DVE (diagonal tiles).
  5. num/den matmuls against [mastery*pad | pad] -> [q, 2] PSUM.
  Every 8 examples: ability = num/den, sigmoid via exp (keeps the ACT Exp
  table resident; no Sigmoid table reload), PE transpose. One output DMA.
"""
import numpy as np

B, S, H, V = 128, 512, 128, 10000
NCORES = 8
EXC = B // NCORES          # examples per core = 16

# e_all / sc column offsets per j-block (packed to keep each matmul
# inside one 2KB PSUM bank): j0 at 0 (512), j1 at 512 (384),
# j3 at 896 (128), j2 at 1024 (256).
OFF = {0: 0, 1: 512, 2: 1024, 3: 896}

_CACHE = {}


def _build_nc():
    import concourse.bacc as bacc
    import concourse.mybir as mybir
    from concourse.tile import TileContext

    f32 = mybir.dt.float32
    bf16 = mybir.dt.bfloat16
    f8 = mybir.dt.float8e4
    AF = mybir.ActivationFunctionType
    ALU = mybir.AluOpType

    nc = bacc.Bacc()

    embs = nc.declare_dram_parameter("embs", [128, EXC * 1024], f8, isOutput=False)
    taux = nc.declare_dram_parameter("taux", [128, EXC * 8], bf16, isOutput=False)
    bnext = nc.declare_dram_parameter("bnext", [128, EXC * 4], f32, isOutput=False)
    biasc = nc.declare_dram_parameter("biasc", [128, 1], f32, isOutput=False)
    causal4 = nc.declare_dram_parameter("causal4", [128, 512], bf16, isOutput=False)
    identf = nc.declare_dram_parameter("identf", [128, 128], f32, isOutput=False)
    out = nc.declare_dram_parameter("out", [EXC, S], f32, isOutput=True)

    with TileContext(nc) as tc:
        with (
            tc.tile_pool(name="psE", bufs=2, space="PSUM") as psE,
            tc.tile_pool(name="psD", bufs=2, space="PSUM") as psD,
            tc.tile_pool(name="persist", bufs=1) as persist,
            tc.tile_pool(name="tts", bufs=6) as tts,
            tc.tile_pool(name="ejs", bufs=3) as ejs,
            tc.tile_pool(name="fin", bufs=2) as fin,
        ):
            # ---------- constants ----------
            # Const DMAs ride the compute engines' DGEs so the sync queue
            # dispatches the embs loads immediately; ACT pre-loads the Exp
            # table during startup dead time (no Sigmoid table is ever
            # needed: the final sigmoid goes through Exp + reciprocal).
            bias_t = persist.tile([128, 1], f32, name="bias_t")
            nc.scalar.dma_start(out=bias_t[:], in_=biasc[:, :])
            dummy = persist.tile([128, 1], f32, name="dummy")
            nc.vector.memset(dummy[:], 0.0)
            dump1 = persist.tile([128, 1], f32, name="dump1")
            nc.scalar.activation(dump1[:], dummy[:], AF.Exp)
            causal_t = persist.tile([128, 512], bf16, name="causal_t")
            nc.gpsimd.dma_start(out=causal_t[:], in_=causal4[:, :])
            identf_t = persist.tile([128, 128], f32, name="identf_t")
            nc.gpsimd.dma_start(out=identf_t[:], in_=identf[:, :])
            taux_t = persist.tile([128, EXC * 8], bf16, name="taux_t")
            nc.gpsimd.dma_start(out=taux_t[:], in_=taux[:, :])
            bnext_t = persist.tile([128, EXC * 4], f32, name="bnext_t")
            nc.gpsimd.dma_start(out=bnext_t[:], in_=bnext[:, :])
            F_all = persist.tile([128, 8 * EXC], f32, name="F_all")
            F3 = F_all[:].rearrange("p (x t) -> p x t", t=2)
            psoAB = psD.tile([32, 256], f32, name="psoAB", tag="pso", bufs=1)
            ogr = persist.tile([32, 256], f32, name="ogr")

            # ---------- main loop (software-pipelined) ----------
            # stage_mm(e): DMA + 4 QK matmuls. stage_rest(e): exp, causal,
            # num/den, copy. Emitting stage_mm(e+1) before stage_rest(e)
            # keeps the PE queue's QK(e+1) ahead of nd(e), so ACT's exp
            # stream is never gated through the previous example's tail.
            def stage_mm(e):
                TT = tts.tile([128, 1024], f8, name="TT", tag="TT")
                nc.sync.dma_start(out=TT[:], in_=embs[:, 1024 * e:1024 * (e + 1)])
                sc = psE.tile([128, 1536], f32, name="sc", tag="sc")
                for j in range(4):
                    n_j = 512 - 128 * j
                    nc.tensor.matmul(
                        sc[:, OFF[j]:OFF[j] + n_j],
                        TT[:, 128 * j:128 * (j + 1)],
                        TT[:, 512 + 128 * j:1024],
                        start=True, stop=True,
                    )
                return sc

            def stage_rest(e, sc):
                # ONE exp over all 1280 score columns (common bias)
                e_all = ejs.tile([128, 1280], bf16, name="e_all", tag="e_all")
                nc.scalar.activation(
                    e_all[:, 0:1280], sc[:, 0:1280], AF.Exp,
                    bias=bias_t[:, 0:1],
                )

                # causal masks on the 4 diagonal tiles (cols {0,512} stride
                # 512 and {896,1024} stride 128), two batched DVE multiplies
                d01 = e_all[:, 0:1024].rearrange("p (b q) -> p b q", b=2)[:, :, 0:128]
                nc.vector.tensor_tensor(
                    out=d01, in0=d01,
                    in1=causal_t[:, 0:256].rearrange("p (b q) -> p b q", b=2),
                    op=ALU.mult,
                )
                d23 = e_all[:, 896:1152].rearrange("p (b q) -> p b q", b=2)
                nc.vector.tensor_tensor(
                    out=d23, in0=d23,
                    in1=causal_t[:, 256:512].rearrange("p (b q) -> p b q", b=2),
                    op=ALU.mult,
                )

                # num/den matmuls: out[q-block c] accumulates over j<=c.
                # Off-diagonal pairs first (they only need exp, not the
                # causal mask) so the PE isn't head-of-line blocked on DVE.
                nd = psD.tile([128, 8], f32, name="nd", tag="nd", bufs=1)
                pairs = [(c, j) for c in (3, 2, 1, 0) for j in range(c + 1)]
                for c, j in pairs:
                    o = OFF[j] + 128 * (c - j)
                    nc.tensor.matmul(
                        nd[:, 2 * c:2 * c + 2],
                        e_all[:, o:o + 128],
                        taux_t[:, 8 * e + 2 * j:8 * e + 2 * j + 2],
                        start=(j == 0), stop=(j == c),
                    )
                nc.vector.tensor_copy(F_all[:, 8 * e:8 * e + 8], nd[:])

            scs = {0: stage_mm(0)}
            for e in range(EXC):
                if e + 1 < EXC:
                    scs[e + 1] = stage_mm(e + 1)
                stage_rest(e, scs.pop(e))

                # ---------- per-group finals (every 8 examples) ----------
                if e % 8 == 7:
                    g = e // 8
                    xs = slice(32 * g, 32 * g + 32)
                    rc_g = fin.tile([128, 32], f32, name="rc_g", tag="rc")
                    nc.vector.reciprocal(rc_g[:], F3[:, xs, 1])
                    at_g = fin.tile([128, 32], f32, name="at_g", tag="at")
                    nc.vector.tensor_tensor(
                        out=at_g[:], in0=F3[:, xs, 0], in1=rc_g[:], op=ALU.mult
                    )
                    zt_g = fin.tile([128, 32], f32, name="zt_g", tag="zt")
                    nc.vector.tensor_tensor(
                        out=zt_g[:], in0=at_g[:], in1=bnext_t[:, xs],
                        op=ALU.subtract,
                    )
                    # sigmoid(z) = 1 / (1 + e^-z), via the resident Exp table
                    ez_g = fin.tile([128, 32], f32, name="ez_g", tag="ez")
                    nc.scalar.activation(ez_g[:], zt_g[:], AF.Exp, scale=-1.0)
                    u_g = fin.tile([128, 32], f32, name="u_g", tag="u")
                    nc.vector.tensor_scalar_add(u_g[:], ez_g[:], 1.0)
                    og_g = fin.tile([128, 32], f32, name="og_g", tag="og")
                    nc.vector.reciprocal(og_g[:], u_g[:])
                    nc.tensor.transpose(
                        psoAB[:, 128 * g:128 * (g + 1)], og_g[:], identf_t[:]
                    )
                    nc.scalar.copy(
                        ogr[:, 128 * g:128 * (g + 1)],
                        psoAB[:, 128 * g:128 * (g + 1)],
                    )

            # ---------- tail: one DMA ----------
            nc.sync.dma_start(
                out=out[:, :].rearrange("(g i1) (i2 p) -> (i1 i2) g p", g=2, i2=4),
                in_=ogr[:].rearrange("i (g p) -> i g p", g=2),
            )

    nc.finalize()
    return nc


def _marshal(inputs):
    import ml_dtypes

    bf16 = ml_dtypes.bfloat16
    f8 = ml_dtypes.float8_e4m3
    hist = np.asarray(inputs["history_items"]).astype(np.int64)
    nxt = np.asarray(inputs["next_items"]).astype(np.int64)
    corrects = np.asarray(inputs["history_corrects"]).astype(np.int64)
    E = np.asarray(inputs["item_embedding"], dtype=np.float32)
    beta = np.asarray(inputs["item_beta_weights"], dtype=np.float32)
    resp = np.asarray(inputs["item_response_vals"], dtype=np.float32)
    k = float(np.asarray(inputs["td_kernel"]).reshape(-1)[0])

    embN = (E / np.linalg.norm(E, axis=1, keepdims=True)).astype(f8)

    p = np.arange(128, dtype=np.float32)
    # common per-partition decay bias: -k*p + 63.5k; each block's constant
    # offset exp(k*(192 - 128j)) is folded into taux below (exact).
    biasc = (k * (63.5 - p)).astype(np.float32).reshape(128, 1)
    blockf = np.exp(np.float64(k) * (192.0 - 128.0 * np.arange(4)))
    causal = (p[:, None] <= p[None, :]).astype(bf16)  # keep s<=q within tile
    causal4 = np.tile(causal, (1, 4))
    identf = np.eye(128, dtype=np.float32)

    # per-example tables
    is_c = (corrects == 2).astype(np.int64)
    mastery = resp[hist, is_c]                       # [B, S]
    pad = (hist != 0).astype(np.float32)             # [B, S]
    mp = (mastery * pad).astype(np.float32)
    bn_full = beta[nxt]                              # [B, S]

    # gathered + transposed normalized embeddings: [B, 128(H), 1024(tok)]
    all_ids = np.concatenate([hist, nxt], axis=1)    # [B, 1024]
    G = embN[all_ids]                                # [B, 1024, 128]
    X = np.ascontiguousarray(G.transpose(0, 2, 1))   # [B, 128, 1024]

    in_maps = []
    for core in range(NCORES):
        embs_c = np.ascontiguousarray(
            X[core * EXC:(core + 1) * EXC].transpose(1, 0, 2).reshape(128, EXC * 1024)
        )
        taux_c = np.zeros((128, EXC * 8), dtype=np.float64)
        bnext_c = np.zeros((128, EXC * 4), dtype=np.float32)
        for e in range(EXC):
            b = core * EXC + e
            mp_b = mp[b].reshape(4, 128).T           # [128(p), 4(j)]
            pad_b = pad[b].reshape(4, 128).T
            for j in range(4):
                taux_c[:, 8 * e + 2 * j] = mp_b[:, j] * blockf[j]
                taux_c[:, 8 * e + 2 * j + 1] = pad_b[:, j] * blockf[j]
            bnext_c[:, 4 * e:4 * e + 4] = bn_full[b].reshape(4, 128).T
        in_maps.append(
            dict(
                embs=embs_c,
                taux=taux_c.astype(bf16),
                bnext=bnext_c,
                biasc=biasc,
                causal4=causal4,
                identf=identf,
            )
        )
    return in_maps


def kernel(**inputs) -> np.ndarray:
    from concourse.bass_utils import run_bass_kernel_spmd

    if "nc" not in _CACHE:
        _CACHE["nc"] = _build_nc()
    nc = _CACHE["nc"]
    in_maps = _marshal(inputs)
    res = run_bass_kernel_spmd(nc, in_maps, list(range(NCORES))).results
    out = np.concatenate([res[c]["out"] for c in range(NCORES)], axis=0)
    return np.ascontiguousarray(out).astype(np.float32)


# revision 23
# speedup vs baseline: 1.1822x; 1.1822x over previous
"""Trainium2 Bass kernel for nn_DPFABase (DPFA knowledge-tracing attention).

Full-input contract: kernel(**inputs) takes the unsharded inputs and returns
the full [B, S] float32 output. Internally: data-parallel over batch across
8 NeuronCores (16 examples per core). Host marshaling (same class as the
beta/response-table prep) pre-normalizes the embedding table, gathers the
per-token rows, and lays them out transposed ([H, token], fp8 e4m3) so the
device kernel spends its time on the actual FLOPs: QK matmuls, softmax,
weighted sums, sigmoid.

Key structure, per example e (16 per core):
  1. One dma_start pulls TT [128(H), 1024] fp8 (cols 0..511 hist_T,
     512..1023 next_T; rows unit-norm).
  2. 4 causal-blocked QK matmuls (fp8) write ONE PSUM tile [128, 1280]
     f32, column-packed [j0:512 | j1:384 | j3:128 | j2:256] so each
     matmul stays inside a 2KB PSUM bank.
  3. ONE ACT Exp over all 1280 cols. The time-decay bias is reduced to a
     single per-partition vector -k*p + 63.5k (common to all blocks) by
     folding each block's constant decay offset exp(k*(192-128j)) into
     the host-marshaled taux columns (exact rescaling; num/den ratio is
     unchanged). Per-q decay parts cancel in softmax.
  4. Two batched causal-mask multiplies on DVE (diagonal tiles).
  5. num/den matmuls against [mastery*pad | pad] -> [q, 2] PSUM.
  Every 8 examples: ability = num/den, sigmoid via exp (keeps the ACT Exp
  table resident; no Sigmoid table reload), PE transpose. One output DMA.
"""
import numpy as np

B, S, H, V = 128, 512, 128, 10000
NCORES = 8
EXC = B // NCORES          # examples per core = 16

# e_all / sc column layout (packed to keep each matmul region inside one
# 2KB PSUM bank): the four diagonal tiles sit contiguously at [0:512]
# (bank 0, one causal-mask op covers them); off-diagonal remainders at
# j0: [512:896], j2: [896:1024], j1: [1024:1280].
OFFD = {0: 0, 1: 128, 2: 256, 3: 384}
OFFO = {0: 512, 1: 1024, 2: 896}

_CACHE = {}


def _build_nc():
    import concourse.bacc as bacc
    import concourse.mybir as mybir
    from concourse.tile import TileContext

    f32 = mybir.dt.float32
    bf16 = mybir.dt.bfloat16
    f8 = mybir.dt.float8e4
    AF = mybir.ActivationFunctionType
    ALU = mybir.AluOpType

    nc = bacc.Bacc()

    embs = nc.declare_dram_parameter("embs", [128, EXC * 1024], f8, isOutput=False)
    taux = nc.declare_dram_parameter("taux", [128, EXC * 8], bf16, isOutput=False)
    bnext = nc.declare_dram_parameter("bnext", [128, EXC * 4], f32, isOutput=False)
    biasc = nc.declare_dram_parameter("biasc", [128, 1], f32, isOutput=False)
    causal4 = nc.declare_dram_parameter("causal4", [128, 512], bf16, isOutput=False)
    identf = nc.declare_dram_parameter("identf", [128, 128], f32, isOutput=False)
    out = nc.declare_dram_parameter("out", [EXC, S], f32, isOutput=True)

    with TileContext(nc) as tc:
        with (
            tc.tile_pool(name="psE", bufs=2, space="PSUM") as psE,
            tc.tile_pool(name="psD", bufs=2, space="PSUM") as psD,
            tc.tile_pool(name="persist", bufs=1) as persist,
            tc.tile_pool(name="tts", bufs=6) as tts,
            tc.tile_pool(name="ejs", bufs=3) as ejs,
            tc.tile_pool(name="fin", bufs=2) as fin,
        ):
            # ---------- constants ----------
            # Const DMAs ride the compute engines' DGEs so the sync queue
            # dispatches the embs loads immediately; ACT pre-loads the Exp
            # table during startup dead time (no Sigmoid table is ever
            # needed: the final sigmoid goes through Exp + reciprocal).
            bias_t = persist.tile([128, 1], f32, name="bias_t")
            nc.scalar.dma_start(out=bias_t[:], in_=biasc[:, :])
            dummy = persist.tile([128, 1], f32, name="dummy")
            nc.vector.memset(dummy[:], 0.0)
            dump1 = persist.tile([128, 1], f32, name="dump1")
            nc.scalar.activation(dump1[:], dummy[:], AF.Exp)
            causal_t = persist.tile([128, 512], bf16, name="causal_t")
            nc.gpsimd.dma_start(out=causal_t[:], in_=causal4[:, :])
            identf_t = persist.tile([128, 128], f32, name="identf_t")
            nc.gpsimd.dma_start(out=identf_t[:], in_=identf[:, :])
            taux_t = persist.tile([128, EXC * 8], bf16, name="taux_t")
            nc.gpsimd.dma_start(out=taux_t[:], in_=taux[:, :])
            bnext_t = persist.tile([128, EXC * 4], f32, name="bnext_t")
            nc.gpsimd.dma_start(out=bnext_t[:], in_=bnext[:, :])
            F_all = persist.tile([128, 8 * EXC], f32, name="F_all")
            F3 = F_all[:].rearrange("p (x t) -> p x t", t=2)
            psoAB = psD.tile([32, 256], f32, name="psoAB", tag="pso", bufs=1)
            ogr = persist.tile([32, 256], f32, name="ogr")

            # ---------- main loop (software-pipelined) ----------
            # stage_mm(e): DMA + 4 QK matmuls. stage_rest(e): exp, causal,
            # num/den, copy. Emitting stage_mm(e+1) before stage_rest(e)
            # keeps the PE queue's QK(e+1) ahead of nd(e), so ACT's exp
            # stream is never gated through the previous example's tail.
            def stage_mm(e):
                TT = tts.tile([128, 1024], f8, name="TT", tag="TT")
                nc.sync.dma_start(out=TT[:], in_=embs[:, 1024 * e:1024 * (e + 1)])
                sc = psE.tile([128, 1536], f32, name="sc", tag="sc")
                for j in range(4):
                    lhsT = TT[:, 128 * j:128 * (j + 1)]
                    # diagonal tile of block j
                    nc.tensor.matmul(
                        sc[:, OFFD[j]:OFFD[j] + 128],
                        lhsT,
                        TT[:, 512 + 128 * j:512 + 128 * (j + 1)],
                        start=True, stop=True,
                    )
                    # off-diagonal remainder of block j (q > diag)
                    if j < 3:
                        n_o = 384 - 128 * j
                        nc.tensor.matmul(
                            sc[:, OFFO[j]:OFFO[j] + n_o],
                            lhsT,
                            TT[:, 512 + 128 * (j + 1):1024],
                            start=True, stop=True,
                        )
                return sc

            def stage_rest(e, sc):
                # ONE exp over all 1280 score columns (common bias)
                e_all = ejs.tile([128, 1280], bf16, name="e_all", tag="e_all")
                nc.scalar.activation(
                    e_all[:, 0:1280], sc[:, 0:1280], AF.Exp,
                    bias=bias_t[:, 0:1],
                )

                # causal mask: the 4 diagonal tiles are contiguous [0:512],
                # one batched DVE bf16 multiply
                nc.vector.tensor_tensor(
                    out=e_all[:, 0:512], in0=e_all[:, 0:512],
                    in1=causal_t[:], op=ALU.mult,
                )

                # num/den matmuls: out[q-block c] accumulates over j<=c.
                # Off-diagonal pairs first (they only need exp, not the
                # causal mask) so the PE isn't head-of-line blocked on DVE.
                nd = psD.tile([128, 8], f32, name="nd", tag="nd", bufs=1)
                pairs = [(c, j) for c in (3, 2, 1, 0) for j in range(c + 1)]
                for c, j in pairs:
                    if c == j:
                        o = OFFD[j]
                    else:
                        o = OFFO[j] + 128 * (c - j - 1)
                    nc.tensor.matmul(
                        nd[:, 2 * c:2 * c + 2],
                        e_all[:, o:o + 128],
                        taux_t[:, 8 * e + 2 * j:8 * e + 2 * j + 2],
                        start=(j == 0), stop=(j == c),
                    )
                nc.vector.tensor_copy(F_all[:, 8 * e:8 * e + 8], nd[:])

            scs = {0: stage_mm(0)}
            for e in range(EXC):
                if e + 1 < EXC:
                    scs[e + 1] = stage_mm(e + 1)
                stage_rest(e, scs.pop(e))

                # ---------- per-group finals (every 8 examples) ----------
                if e % 8 == 7:
                    g = e // 8
                    xs = slice(32 * g, 32 * g + 32)
                    rc_g = fin.tile([128, 32], f32, name="rc_g", tag="rc")
                    nc.vector.reciprocal(rc_g[:], F3[:, xs, 1])
                    at_g = fin.tile([128, 32], f32, name="at_g", tag="at")
                    nc.vector.tensor_tensor(
                        out=at_g[:], in0=F3[:, xs, 0], in1=rc_g[:], op=ALU.mult
                    )
                    zt_g = fin.tile([128, 32], f32, name="zt_g", tag="zt")
                    nc.vector.tensor_tensor(
                        out=zt_g[:], in0=at_g[:], in1=bnext_t[:, xs],
                        op=ALU.subtract,
                    )
                    # sigmoid(z) = 1 / (1 + e^-z), via the resident Exp table
                    ez_g = fin.tile([128, 32], f32, name="ez_g", tag="ez")
                    nc.scalar.activation(ez_g[:], zt_g[:], AF.Exp, scale=-1.0)
                    u_g = fin.tile([128, 32], f32, name="u_g", tag="u")
                    nc.vector.tensor_scalar_add(u_g[:], ez_g[:], 1.0)
                    og_g = fin.tile([128, 32], f32, name="og_g", tag="og")
                    nc.vector.reciprocal(og_g[:], u_g[:])
                    nc.tensor.transpose(
                        psoAB[:, 128 * g:128 * (g + 1)], og_g[:], identf_t[:]
                    )
                    nc.scalar.copy(
                        ogr[:, 128 * g:128 * (g + 1)],
                        psoAB[:, 128 * g:128 * (g + 1)],
                    )

            # ---------- tail: one DMA ----------
            nc.sync.dma_start(
                out=out[:, :].rearrange("(g i1) (i2 p) -> (i1 i2) g p", g=2, i2=4),
                in_=ogr[:].rearrange("i (g p) -> i g p", g=2),
            )

    nc.finalize()
    return nc


def _marshal(inputs):
    import ml_dtypes

    bf16 = ml_dtypes.bfloat16
    f8 = ml_dtypes.float8_e4m3
    hist = np.asarray(inputs["history_items"]).astype(np.int64)
    nxt = np.asarray(inputs["next_items"]).astype(np.int64)
    corrects = np.asarray(inputs["history_corrects"]).astype(np.int64)
    E = np.asarray(inputs["item_embedding"], dtype=np.float32)
    beta = np.asarray(inputs["item_beta_weights"], dtype=np.float32)
    resp = np.asarray(inputs["item_response_vals"], dtype=np.float32)
    k = float(np.asarray(inputs["td_kernel"]).reshape(-1)[0])

    embN = (E / np.linalg.norm(E, axis=1, keepdims=True)).astype(f8)

    p = np.arange(128, dtype=np.float32)
    # common per-partition decay bias: -k*p + 63.5k; each block's constant
    # offset exp(k*(192 - 128j)) is folded into taux below (exact).
    biasc = (k * (63.5 - p)).astype(np.float32).reshape(128, 1)
    blockf = np.exp(np.float64(k) * (192.0 - 128.0 * np.arange(4)))
    causal = (p[:, None] <= p[None, :]).astype(bf16)  # keep s<=q within tile
    causal4 = np.tile(causal, (1, 4))
    identf = np.eye(128, dtype=np.float32)

    # per-example tables
    is_c = (corrects == 2).astype(np.int64)
    mastery = resp[hist, is_c]                       # [B, S]
    pad = (hist != 0).astype(np.float32)             # [B, S]
    mp = (mastery * pad).astype(np.float32)
    bn_full = beta[nxt]                              # [B, S]

    # gathered + transposed normalized embeddings: [B, 128(H), 1024(tok)]
    all_ids = np.concatenate([hist, nxt], axis=1)    # [B, 1024]
    G = embN[all_ids]                                # [B, 1024, 128]
    X = np.ascontiguousarray(G.transpose(0, 2, 1))   # [B, 128, 1024]

    in_maps = []
    for core in range(NCORES):
        embs_c = np.ascontiguousarray(
            X[core * EXC:(core + 1) * EXC].transpose(1, 0, 2).reshape(128, EXC * 1024)
        )
        taux_c = np.zeros((128, EXC * 8), dtype=np.float64)
        bnext_c = np.zeros((128, EXC * 4), dtype=np.float32)
        for e in range(EXC):
            b = core * EXC + e
            mp_b = mp[b].reshape(4, 128).T           # [128(p), 4(j)]
            pad_b = pad[b].reshape(4, 128).T
            for j in range(4):
                taux_c[:, 8 * e + 2 * j] = mp_b[:, j] * blockf[j]
                taux_c[:, 8 * e + 2 * j + 1] = pad_b[:, j] * blockf[j]
            bnext_c[:, 4 * e:4 * e + 4] = bn_full[b].reshape(4, 128).T
        in_maps.append(
            dict(
                embs=embs_c,
                taux=taux_c.astype(bf16),
                bnext=bnext_c,
                biasc=biasc,
                causal4=causal4,
                identf=identf,
            )
        )
    return in_maps


def kernel(**inputs) -> np.ndarray:
    from concourse.bass_utils import run_bass_kernel_spmd

    if "nc" not in _CACHE:
        _CACHE["nc"] = _build_nc()
    nc = _CACHE["nc"]
    in_maps = _marshal(inputs)
    res = run_bass_kernel_spmd(nc, in_maps, list(range(NCORES))).results
    out = np.concatenate([res[c]["out"] for c in range(NCORES)], axis=0)
    return np.ascontiguousarray(out).astype(np.float32)
